# revision 37
# baseline (speedup 1.0000x reference)
"""Trainium2 Bass kernel for nn_ConditionedLM: transposed feature-major layout, strip-major projections, decode interleaved into the LM recurrence."""

import numpy as np
import ml_dtypes
from contextlib import ExitStack

import concourse.bass as bass
import concourse.mybir as mybir
import concourse.tile as tile
from concourse import bacc
from concourse.bass_utils import run_bass_kernel_spmd
from concourse.masks import make_identity

dt = mybir.dt
bf16 = ml_dtypes.bfloat16

V, E, He, H = 50257, 512, 512, 1024
B, T, Lt = 16, 128, 64
NCORES = 8
VSTRIPS = 50
VS = VSTRIPS * 128
N_TOK = B * T
N_TAB = B * Lt

MM_DT = dt.bfloat16
MM_NP = bf16
ACT = mybir.ActivationFunctionType

_CACHE = {}


def _gate_perm(h):
    return np.concatenate([np.arange(0, h), np.arange(h, 2 * h),
                           np.arange(3 * h, 4 * h), np.arange(2 * h, 3 * h)])


def build_bass():
    nc = bacc.Bacc()

    embT_d = nc.dram_tensor("embT", [128, (E // 128) * N_TOK], MM_DT,
                            kind="ExternalInput")
    tembT_d = nc.dram_tensor("tembT", [128, (E // 128) * N_TAB], MM_DT,
                             kind="ExternalInput")
    tembTr_d = nc.dram_tensor("tembTr", [128, (E // 128) * N_TAB], MM_DT,
                              kind="ExternalInput")
    wih_enc_d = nc.dram_tensor("wih_enc_t", [E, 4096], MM_DT, kind="ExternalInput")
    wih_lm_d = nc.dram_tensor("wih_lm_t", [E, 4096], MM_DT, kind="ExternalInput")
    whh_enc_d = nc.dram_tensor("whh_enc_t", [He, 4096], MM_DT, kind="ExternalInput")
    whh_lm_d = nc.dram_tensor("whh_lm_t", [H, 4096], MM_DT, kind="ExternalInput")
    b_enc_d = nc.dram_tensor("b_enc_s", [128, 32], dt.float32, kind="ExternalInput")
    b_lm_d = nc.dram_tensor("b_lm_s", [128, 32], dt.float32, kind="ExternalInput")
    h0T_d = nc.dram_tensor("h0T", [128, 128], MM_DT, kind="ExternalInput")
    c0T_d = nc.dram_tensor("c0T", [128, 128], MM_DT, kind="ExternalInput")
    wdec_d = nc.dram_tensor("wdec_t", [128, VSTRIPS * 8 * 128], MM_DT,
                            kind="ExternalInput")
    bdec_d = nc.dram_tensor("bdec_s", [128, VSTRIPS], dt.float32,
                            kind="ExternalInput")
    out_d = nc.dram_tensor("out", [VS, N_TOK], MM_DT, kind="ExternalOutput")

    with tile.TileContext(nc) as tc, ExitStack() as ctx:
        dram = ctx.enter_context(tc.tile_pool(name="dram", bufs=1, space="DRAM"))
        xp_enc_d = dram.tile([32, 128, N_TAB], MM_DT)
        xp_lm_d = dram.tile([32, 128, N_TOK], MM_DT)

        const = ctx.enter_context(tc.tile_pool(name="const", bufs=1))
        ident = const.tile([128, 128], MM_DT)
        make_identity(nc, ident[:])
        bias_p = ctx.enter_context(tc.tile_pool(name="bias", bufs=1))
        b_enc_sb = bias_p.tile([128, 32], dt.float32)
        b_lm_sb = bias_p.tile([128, 32], dt.float32)
        bdec_sb = bias_p.tile([128, VSTRIPS], dt.float32)
        nc.sync.dma_start(b_enc_sb[:], b_enc_d[:])
        nc.sync.dma_start(b_lm_sb[:], b_lm_d[:])
        nc.sync.dma_start(bdec_sb[:], bdec_d[:])

        ysT_p = ctx.enter_context(tc.tile_pool(name="ysT", bufs=1))
        ysT = ysT_p.tile([128, 8, 16 * (T + 1)], MM_DT)
        state = ctx.enter_context(tc.tile_pool(name="state", bufs=2))
        chp = ctx.enter_context(tc.tile_pool(name="chain", bufs=2))
        xq_p = ctx.enter_context(tc.tile_pool(name="xq", bufs=2))
        psum_g = ctx.enter_context(tc.tile_pool(name="psum_g", bufs=2, space="PSUM"))
        psum_mm = ctx.enter_context(
            tc.tile_pool(name="psum_mm", bufs=3, space="PSUM"))

        rotq = [nc.sync, nc.scalar, nc.gpsimd]

        with tc.tile_pool(name="gather", bufs=1) as gpool, \
             tc.tile_pool(name="wih", bufs=1) as wpool, \
             tc.tile_pool(name="xpout", bufs=3) as xpo:

            def proj_strips(wih_sb, eT_sb, n_tok, srange, bias_sb, xp_base, tag):
                for s in srange:
                    xog = xpo.tile([128, n_tok], MM_DT, tag=tag, name="xog")
                    for blk in range(n_tok // 512):
                        px = psum_mm.tile([128, 512], dt.float32, tag="mm")
                        for k in range(4):
                            nc.tensor.matmul(
                                px[:], wih_sb[:, k, 128 * s:128 * (s + 1)],
                                eT_sb[:, k, 512 * blk:512 * (blk + 1)],
                                start=(k == 0), stop=(k == 3))
                        nc.scalar.activation(
                            xog[:, 512 * blk:512 * (blk + 1)], px[:],
                            ACT.Identity, bias=bias_sb[:, s:s + 1])
                    rotq[s % 3].dma_start(xp_base[s], xog[:])

            wih_enc_sb = wpool.tile([128, 4, 4096], MM_DT, tag="w", name="we")
            for k in range(4):
                nc.sync.dma_start(wih_enc_sb[:, k, :],
                                  wih_enc_d[128 * k:128 * (k + 1), :])
            tembT = gpool.tile([128, 4, N_TAB], MM_DT, tag="e", name="te")
            nc.sync.dma_start(tembT[:], tembT_d[:])
            proj_strips(wih_enc_sb, tembT, N_TAB, range(0, 16), b_enc_sb,
                        xp_enc_d, "xoe")
            tembTr = gpool.tile([128, 4, N_TAB], MM_DT, tag="e", name="tr")
            nc.scalar.dma_start(tembTr[:], tembTr_d[:])
            proj_strips(wih_enc_sb, tembTr, N_TAB, range(16, 32), b_enc_sb,
                        xp_enc_d, "xoe")
            embT = gpool.tile([128, 4, N_TOK], MM_DT, tag="e2", name="em")
            for k in range(4):
                nc.gpsimd.dma_start(embT[:, k, :],
                                    embT_d[:, N_TOK * k:N_TOK * (k + 1)])
            wih_lm_sb = wpool.tile([128, 4, 4096], MM_DT, tag="w", name="wl")
            for k in range(4):
                nc.scalar.dma_start(wih_lm_sb[:, k, :],
                                    wih_lm_d[128 * k:128 * (k + 1), :])
            proj_strips(wih_lm_sb, embT, N_TOK, range(0, 32), b_lm_sb,
                        xp_lm_d, "xo")

        def load_xq(xp_base, q):
            xq = xq_p.tile([128, 32, 512], MM_DT, tag="xq", name="xq")
            for si in range(32):
                rotq[si % 3].dma_start(
                    xq[:, si, :], xp_base[si, :, 512 * q:512 * (q + 1)])
            return xq

        xh = [load_xq(xp_enc_d, 0), load_xq(xp_enc_d, 1)]

        whh_enc_ctx = tc.tile_pool(name="whh_enc", bufs=1)
        whh_enc_p = whh_enc_ctx.__enter__()
        whh_enc = whh_enc_p.tile([128, 4, 4096], MM_DT)
        for k in range(4):
            nc.sync.dma_start(whh_enc[:, k, :],
                              whh_enc_d[128 * k:128 * (k + 1), :])

        h_cur = state.tile([128, 128], MM_DT, tag="h")
        c_cur = state.tile([128, 128], MM_DT, tag="c")
        nc.sync.dma_start(h_cur[:], h0T_d[:])
        nc.sync.dma_start(c_cur[:], c0T_d[:])

        def enc_inject(pg, st):
            # ONE full-tile inject: start=True zeroes a whole 2KB PSUM region,
            # so per-slice starts would wipe earlier strips' values
            xq = xh[st // 32]
            c0 = 16 * (st % 32)
            nc.tensor.matmul(pg[:], ident[:], xq[:, :, c0:c0 + 16],
                             start=True, stop=False, skip_group_check=True)

        def enc_gates(pg, h, srange):
            for s in srange:
                hoff = 64 if s >= 16 else 0
                for k in range(4):
                    nc.tensor.matmul(
                        pg[:, 16 * s:16 * (s + 1)],
                        whh_enc[:, k, 128 * s:128 * (s + 1)],
                        h[:, hoff + 16 * k:hoff + 16 * (k + 1)],
                        start=False, stop=(k == 3), skip_group_check=True)

        pg = psum_g.tile([128, 512], dt.float32, tag="pg")
        enc_inject(pg, 0)
        enc_gates(pg, h_cur, range(32))

        lm_q = {}
        for st in range(Lt):
            last = st == Lt - 1
            sigb = chp.tile([128, 512], MM_DT, tag="sig")
            cn = state.tile([128, 128], MM_DT, tag="c")
            hn = state.tile([128, 128], MM_DT, tag="h")
            tcn = chp.tile([128, 128], MM_DT, tag="tcn")
            t1 = chp.tile([128, 128], MM_DT, tag="t1")
            t2 = chp.tile([128, 128], MM_DT, tag="t2")
            if st == 32:
                lm_q[0] = load_xq(xp_lm_d, 0)
            if not last:
                pg_n = psum_g.tile([128, 512], dt.float32, tag="pg")
                enc_inject(pg_n, st + 1)
            for d, off, coff in ((0, 0, 0), (1, 256, 64)):
                sg = slice(off, off + 192)
                nc.scalar.activation(sigb[:, sg], pg[:, sg], ACT.Sigmoid)
                gg = slice(off + 192, off + 256)
                nc.scalar.activation(sigb[:, gg], pg[:, gg], ACT.Tanh)
                cs = slice(coff, coff + 64)
                nc.vector.tensor_mul(t1[:, cs], sigb[:, off + 64:off + 128],
                                     c_cur[:, cs])
                nc.vector.tensor_mul(t2[:, cs], sigb[:, off:off + 64],
                                     sigb[:, gg])
                nc.vector.tensor_add(cn[:, cs], t1[:, cs], t2[:, cs])
                nc.scalar.activation(tcn[:, cs], cn[:, cs], ACT.Tanh)
                nc.vector.tensor_mul(hn[:, cs], sigb[:, off + 128:off + 192],
                                     tcn[:, cs])
                if not last:
                    enc_gates(pg_n, hn,
                              range(0, 16) if d == 0 else range(16, 32))
            c_cur, h_cur = cn, hn
            if not last:
                pg = pg_n

        # Phase D
        c_lm = state.tile([128, 128], MM_DT, tag="c")
        for khalf in range(2):
            for bhalf in range(2):
                for k in range(4):
                    s0 = 64 * bhalf + khalf + 16 * k
                    nc.sync.dma_start(
                        ysT[:, 4 * khalf + k, 8 * bhalf:8 * bhalf + 8],
                        h_cur[:, s0:s0 + 15:2])
                    d0 = 16 * (4 * khalf + k) + 8 * bhalf
                    nc.sync.dma_start(c_lm[:, d0:d0 + 8],
                                      c_cur[:, s0:s0 + 15:2])
        c_cur = c_lm

        whh_enc_ctx.__exit__(None, None, None)

        # Phase E
        whh_lm_ctx = tc.tile_pool(name="whh_lm", bufs=1)
        whh_lm_p = whh_lm_ctx.__enter__()
        whh_lm = whh_lm_p.tile([128, 8, 4096], MM_DT)
        for k in range(8):
            (nc.sync if k % 2 == 0 else nc.gpsimd).dma_start(
                whh_lm[:, k, :], whh_lm_d[128 * k:128 * (k + 1), :])

        wdp_ctx = tc.tile_pool(name="wdec", bufs=6)
        wdp = wdp_ctx.__enter__()
        dop_ctx = tc.tile_pool(name="dout", bufs=4)
        dop = dop_ctx.__enter__()
        dec_q = [(sv, 0) for sv in range(VSTRIPS)] + \
                [(sv, 1) for sv in range(VSTRIPS)] + \
                [(sv, b) for sv in range(VSTRIPS) for b in (2, 3)]
        wn_cur = [None, None]

        def emit_dec(sv, blk, qi):
            if wn_cur[0] != sv:
                wn = wdp.tile([128, 8, 128], MM_DT, tag="wn")
                rotq[qi % 3].dma_start(
                    wn[:], wdec_d[:, 1024 * sv:1024 * (sv + 1)])
                wn_cur[0], wn_cur[1] = sv, wn
            wn = wn_cur[1]
            pd = psum_mm.tile([128, 512], dt.float32, tag="mm")
            for k in range(8):
                nc.tensor.matmul(
                    pd[:], wn[:, k, :],
                    ysT[:, k, 16 + 512 * blk:16 + 512 * (blk + 1)],
                    start=(k == 0), stop=(k == 7))
            ob = dop.tile([128, 512], MM_DT, tag="ob")
            nc.scalar.activation(ob[:], pd[:], ACT.Identity,
                                 bias=bdec_sb[:, sv:sv + 1])
            rotq[(qi + 1) % 3].dma_start(
                out_d[128 * sv:128 * (sv + 1),
                      512 * blk:512 * (blk + 1)], ob[:])

        def lm_gates(pg, hslot, ks):
            for s in range(32):
                for k in ks:
                    nc.tensor.matmul(
                        pg[:, 16 * s:16 * (s + 1)],
                        whh_lm[:, k, 128 * s:128 * (s + 1)],
                        ysT[:, k, 16 * hslot:16 * (hslot + 1)],
                        start=False, stop=(k == 7), skip_group_check=True)

        def lm_inject(pg, t):
            # ONE full-tile inject (see enc_inject: PSUM zero-region is 2KB)
            xq = lm_q[t // 32]
            c0 = 16 * (t % 32)
            nc.tensor.matmul(pg[:], ident[:], xq[:, :, c0:c0 + 16],
                             start=True, stop=False, skip_group_check=True)

        pg = psum_g.tile([128, 512], dt.float32, tag="pg")
        lm_inject(pg, 0)
        lm_gates(pg, 0, range(8))

        for t in range(T):
            last = t == T - 1
            sigb = chp.tile([128, 512], MM_DT, tag="sig")
            cn = state.tile([128, 128], MM_DT, tag="c")
            tcn = chp.tile([128, 128], MM_DT, tag="tcn")
            t1 = chp.tile([128, 128], MM_DT, tag="t1")
            t2 = chp.tile([128, 128], MM_DT, tag="t2")
            if t % 32 == 0 and t // 32 + 1 <= 3:
                lm_q[t // 32 + 1] = load_xq(xp_lm_d, t // 32 + 1)
            if not last:
                pg_n = psum_g.tile([128, 512], dt.float32, tag="pg")
                lm_inject(pg_n, t + 1)
            for ch in range(2):
                co = 64 * ch
                cs = slice(co, co + 64)
                pgv = pg[:, 0:384].rearrange("p (g c) -> p g c", g=3)
                sgv = sigb[:, 0:384].rearrange("p (g c) -> p g c", g=3)
                nc.scalar.activation(sgv[:, :, cs], pgv[:, :, cs], ACT.Sigmoid)
                nc.scalar.activation(sigb[:, 384 + co:448 + co],
                                     pg[:, 384 + co:448 + co], ACT.Tanh)
                nc.vector.tensor_mul(t1[:, cs], sigb[:, 128 + co:192 + co],
                                     c_cur[:, cs])
                nc.vector.tensor_mul(t2[:, cs], sigb[:, co:64 + co],
                                     sigb[:, 384 + co:448 + co])
                nc.vector.tensor_add(cn[:, cs], t1[:, cs], t2[:, cs])
                nc.scalar.activation(tcn[:, cs], cn[:, cs], ACT.Tanh)
                hv = ysT[:, 4 * ch:4 * (ch + 1), 16 * (t + 1):16 * (t + 2)]
                nc.vector.tensor_mul(
                    hv, sigb[:, 256 + co:320 + co].rearrange(
                        "p (k b) -> p k b", k=4),
                    tcn[:, cs].rearrange("p (k b) -> p k b", k=4))
                if not last:
                    lm_gates(pg_n, t + 1, range(4 * ch, 4 * ch + 4))
            if dec_q and t >= 32 * (dec_q[0][1] + 1) + 1:
                emit_dec(*dec_q.pop(0), qi=t)
            c_cur = cn
            if not last:
                pg = pg_n

        for i, (sv, blk) in enumerate(dec_q):
            emit_dec(sv, blk, qi=i)
        dop_ctx.__exit__(None, None, None)
        wdp_ctx.__exit__(None, None, None)
        whh_lm_ctx.__exit__(None, None, None)

    nc.compile()
    return nc


def _embT_host(tbl, idx):
    g = np.asarray(tbl, np.float32)[idx]
    n = g.shape[0]
    gt = g.T.reshape(E // 128, 128, n)
    return np.ascontiguousarray(
        gt.transpose(1, 0, 2).reshape(128, -1)).astype(MM_NP)


def _strip_bias(b):
    return np.ascontiguousarray(b.reshape(-1, 128).T).astype(np.float32)


def _prep_inputs(inputs):
    f32 = np.float32
    x = np.asarray(inputs["x"]).astype(np.int64)
    table = np.asarray(inputs["table"]).astype(np.int64)
    xf = x.T.reshape(-1)
    tf = table.T.reshape(-1)
    tr = table[:, ::-1].T.reshape(-1)

    pe = _gate_perm(He)
    pl = _gate_perm(H)
    wih_enc_t = np.concatenate(
        [np.asarray(inputs["Wih_f"])[pe].T, np.asarray(inputs["Wih_b"])[pe].T],
        axis=1).astype(MM_NP)
    whh_enc_t = np.concatenate(
        [np.asarray(inputs["Whh_f"])[pe].T, np.asarray(inputs["Whh_b"])[pe].T],
        axis=1).astype(MM_NP)
    wih_lm_t = np.ascontiguousarray(np.asarray(inputs["Wih_lm"])[pl].T).astype(MM_NP)
    whh_lm_t = np.ascontiguousarray(np.asarray(inputs["Whh_lm"])[pl].T).astype(MM_NP)
    b_enc = np.concatenate([np.asarray(inputs["b_f"])[pe],
                            np.asarray(inputs["b_b"])[pe]])
    b_lm = np.asarray(inputs["b_lm"])[pl]

    def init_T(v):
        v = np.asarray(v, f32)
        o = np.zeros((128, 128), f32)
        for d in range(2):
            for k in range(4):
                o[:, 64 * d + 16 * k:64 * d + 16 * (k + 1)] = \
                    v[d, :, 128 * k:128 * (k + 1)].T
        return o.astype(MM_NP)

    wdec = np.asarray(inputs["Wdec"]).astype(f32)
    bdec = np.asarray(inputs["bdec"]).astype(f32)
    wdec_pad = np.zeros((NCORES * VS, H), f32)
    wdec_pad[:V] = wdec
    bdec_pad = np.zeros(NCORES * VS, f32)
    bdec_pad[:V] = bdec

    common = dict(
        embT=_embT_host(inputs["embed"], xf),
        tembT=_embT_host(inputs["table_embed"], tf),
        tembTr=_embT_host(inputs["table_embed"], tr),
        wih_enc_t=wih_enc_t, wih_lm_t=wih_lm_t,
        whh_enc_t=whh_enc_t, whh_lm_t=whh_lm_t,
        b_enc_s=_strip_bias(b_enc), b_lm_s=_strip_bias(b_lm),
        h0T=init_T(inputs["enc_h0"]), c0T=init_T(inputs["enc_c0"]),
    )
    in_maps = []
    for c in range(NCORES):
        m = dict(common)
        wc = wdec_pad[c * VS:(c + 1) * VS].reshape(VSTRIPS, 128, 8, 128)
        m["wdec_t"] = np.ascontiguousarray(
            wc.transpose(3, 0, 2, 1).reshape(128, -1)).astype(MM_NP)
        m["bdec_s"] = _strip_bias(bdec_pad[c * VS:(c + 1) * VS])
        in_maps.append(m)
    return in_maps


def kernel(**inputs) -> np.ndarray:
    import time as _time
    if "nc" not in _CACHE:
        _CACHE["nc"] = build_bass()
    nc = _CACHE["nc"]
    in_maps = _prep_inputs(inputs)
    res = None
    for attempt in range(3):
        try:
            res = run_bass_kernel_spmd(nc, in_maps, core_ids=list(range(NCORES)))
            break
        except Exception:
            if attempt == 2:
                raise
            _time.sleep(10)
    outs = [np.asarray(res.results[c]["out"], np.float32) for c in range(NCORES)]
    full = np.concatenate(outs, axis=0)[:V]
    return np.ascontiguousarray(full.T.reshape(T, B, V))
